# revision 1
# baseline (speedup 1.0000x reference)
"""ClusterLoss Trainium2 kernel: 8-core data-parallel Bass/Tile implementation.

Math (C=64 classes, D=192, N=262144):
  sums[c]  = sum_{i: lab_i=c} x_i            (one-hot matmul, PSUM accumulate)
  means    = sums / counts                   (counts via host bincount)
  intra    = sum_i ||x_i - means[lab_i] + eps||_2
  inter    = sum_{i != j} ||mean_i - mean_j + eps||_2
  out      = intra - inter
8 cores shard rows; class sums are AllReduce'd; inter replicated; host
sums the 8 partial scalars.

v2 design vs baseline:
  - host supplies x in bf16 (halves HBM) and both one-hot layouts
    (normal for the sums matmul lhsT, transposed for the phase-2 gather
    lhsT) - removes all DVE one-hot builds / transposes / copies.
  - phase 2 splits work across scalar+vector+gpsimd via three paths:
      P1: gather+ident matmul -> ACT.Square -> ts-accum reduce
      P2: gather matmul only  -> DVE stt (diff) -> stt-square-accum
      P3: gather+ident matmul -> DVE ts copy   -> stt-square-accum
  - dummy matmuls keep the PE HAM-warm across the AllReduce gap.

eps note: intra additive eps shifts result ~1e-10 relative - dropped.
Inter eps kept exactly via the Gram expansion (see inter block).
"""

import contextlib

import numpy as np

N, D, C, W = 262144, 192, 64, 8
NL = N // W            # rows per core = 32768
T = NL // 128          # 128-row tiles per core = 256
M_IT = T // 2          # tile pairs = 128
EPS = 1e-6

_COMPILED = {}

# phase-2 path mix (counts over M_IT pairs) and reduce-engine choice
N_WARM = 0  # PE warm-keepers (scheduler hoists them into phase 1: off)


def _mk_paths():
    """Blocks of 28 scalar-path pairs (7 aligned 4-pair reduce chunks)
    followed by 4 DVE-path pairs, repeated 4x over the 128 pairs."""
    paths = []
    for g in range(4):
        paths += [1] * 28 + [2] * 4
    return paths


def _build(debug=False):
    import sys
    if "/opt/trn_rl_repo" not in sys.path:
        sys.path.insert(0, "/opt/trn_rl_repo")
    from concourse import bacc, tile, mybir

    f32 = mybir.dt.float32
    bf16 = mybir.dt.bfloat16
    ACT = mybir.ActivationFunctionType
    ALU = mybir.AluOpType

    nc = bacc.Bacc("TRN2", target_bir_lowering=False, debug=debug,
                   num_devices=W)

    x_d = nc.dram_tensor("x", [128, T * D], bf16, kind="ExternalInput")
    ohn_d = nc.dram_tensor("ohn", [128, M_IT * 128], bf16,
                           kind="ExternalInput")
    oht_d = nc.dram_tensor("oht", [128, M_IT * 128], bf16,
                           kind="ExternalInput")
    identb_d = nc.dram_tensor("identb", [128, 128], bf16,
                              kind="ExternalInput")
    identf_d = nc.dram_tensor("identf", [128, 128], f32,
                              kind="ExternalInput")
    ninv_d = nc.dram_tensor("ninv", [128, 1], f32, kind="ExternalInput")
    inv_d = nc.dram_tensor("inv", [64, 1], f32, kind="ExternalInput")
    ones_d = nc.dram_tensor("ones", [128, 1], f32, kind="ExternalInput")
    offdiag_d = nc.dram_tensor("offdiag", [C, C], f32, kind="ExternalInput")
    foldm_d = nc.dram_tensor("foldm", [128, C], f32, kind="ExternalInput")
    out_d = nc.dram_tensor("out", [1, 1], f32, kind="ExternalOutput")

    paths = _mk_paths()

    with tile.TileContext(nc) as tc:
        with (
            tc.tile_pool(name="singles", bufs=1) as sg,
            tc.tile_pool(name="scr", bufs=6) as scr_p,
            tc.tile_pool(name="dram", bufs=1, space="DRAM") as dram_p,
        ):
            # ---- persistent SBUF ----
            xbf = sg.tile([128, T * D], bf16)         # resident logits
            ohn = sg.tile([128, M_IT * 128], bf16)    # pair-normal one-hots
            oht = sg.tile([128, M_IT * 128], bf16)    # pair-transposed
            identb = sg.tile([128, 128], bf16)
            identf = sg.tile([128, 128], f32)
            ninv = sg.tile([128, 1], f32)
            inv = sg.tile([64, 1], f32)
            ones = sg.tile([128, 1], f32)
            offdiag = sg.tile([C, C], f32)
            foldm = sg.tile([128, C], f32)
            sums_sb = sg.tile([128, D], f32)
            mneW = sg.tile([128, 2 * D], bf16)        # block-diag -means
            means_f = sg.tile([64, D], f32)
            sqsum = sg.tile([128, T], f32)
            norms = sg.tile([128, T], bf16)
            nsum = sg.tile([128, 1], f32)
            A1 = sg.tile([96, 64], f32)
            A2 = sg.tile([96, 64], f32)
            B1 = sg.tile([96, 64], f32)
            B2 = sg.tile([96, 64], f32)
            rr = sg.tile([64, 1], f32)
            ts = sg.tile([64, 1], f32)
            vb = sg.tile([64, 1], f32)
            uu = sg.tile([64, 1], f32)
            ut_sb = sg.tile([1, 64], f32)
            ones_row = sg.tile([1, 64], f32)
            sums_stage = sg.tile([64, D], f32)
            sums_stage2 = sg.tile([64, D], f32)
            sAx = sg.tile([128, D], f32)
            sBx = sg.tile([128, D], f32)
            dists = sg.tile([64, 64], f32)
            dscr = sg.tile([64, 64], f32)
            inter_rows = sg.tile([64, 1], f32)
            rscr = sg.tile([64, D], bf16)
            wmix = sg.tile([128, D], f32)
            tmp_s = sg.tile([1, 1], f32)
            outv = sg.tile([1, 1], f32)

            # ---- input DMA: x on sync+gpsimd queues, one-hots behind ----
            # small leading chunks so pair-0 data lands early, then bulk
            xsplits = [0, 4, 8, 12, 16] + list(range(32, T + 1, 16))
            for ci in range(len(xsplits) - 1):
                lo, hi = xsplits[ci] * D, xsplits[ci + 1] * D
                q = nc.sync if ci % 2 == 0 else nc.gpsimd
                q.dma_start(xbf[:, lo:hi], x_d.ap()[:, lo:hi])
            nc.scalar.dma_start(foldm[:], foldm_d.ap())
            osplits = [0, 2, 4, 8, 16] + list(range(32, M_IT + 1, 16))
            for ci in range(len(osplits) - 1):
                lo, hi = osplits[ci] * 128, osplits[ci + 1] * 128
                nc.scalar.dma_start(ohn[:, lo:hi], ohn_d.ap()[:, lo:hi])
            nc.scalar.dma_start(identb[:], identb_d.ap())
            nc.scalar.dma_start(identf[:], identf_d.ap())
            nc.scalar.dma_start(ninv[:], ninv_d.ap())
            nc.scalar.dma_start(inv[:], inv_d.ap())
            nc.scalar.dma_start(ones[:], ones_d.ap())
            nc.scalar.dma_start(offdiag[:], offdiag_d.ap())
            # oht is needed only in phase 2: queue it behind ohn on the
            # scalar ring (keeps the gpsimd ring free for the AR copies).
            OCH = 16 * 128
            for ci in range(M_IT // 16):
                lo, hi = ci * OCH, (ci + 1) * OCH
                nc.scalar.dma_start(oht[:, lo:hi], oht_d.ap()[:, lo:hi])
            nc.vector.memset(mneW[:], 0.0)

            # ---- phase 1: class sums via one-hot matmuls ----
            # lhsT = ohn pair [128,128] (cols = 2x64 classes), rhs = x pair
            # [128,384]. Diagonal blocks of the wide PSUM are the two
            # partial sums; fold extracts+adds them.
            p1_ctx = contextlib.ExitStack()
            ps_sums = p1_ctx.enter_context(
                tc.tile_pool(name="ps_sums", bufs=1, space="PSUM"))
            ps_fold = p1_ctx.enter_context(
                tc.tile_pool(name="ps_fold", bufs=2, space="PSUM"))
            QM = M_IT // 4
            wides = [ps_sums.tile([128, 2 * D], f32, tag=f"w{k}",
                                  name=f"wide{k}")
                     for k in range(4)]
            stages = [sums_stage, sums_stage2,
                      sg.tile([64, D], f32, name="sums_stage3"),
                      sg.tile([64, D], f32, name="sums_stage4")]

            def fold(wide, stage_out):
                nc.vector.tensor_copy(wmix[0:64, :], wide[0:64, 0:D])
                nc.vector.tensor_copy(wmix[64:128, :], wide[64:128, D:2 * D])
                fps = ps_fold.tile([64, D], f32, tag="fold")
                nc.tensor.matmul(fps[:], foldm[:], wmix[:],
                                 start=True, stop=True)
                nc.vector.tensor_copy(stage_out[:], fps[:])

            ar_outs = []

            def launch_ar(src, k):
                ar_in = dram_p.tile([64, D], f32, tag=f"ar{k}i",
                                    name=f"arin{k}")
                ar_out = dram_p.tile([64, D], f32, tag=f"ar{k}o",
                                     addr_space="Shared", name=f"arout{k}")
                nc.gpsimd.dma_start(ar_in[:], src[:])
                nc.gpsimd.collective_compute(
                    "AllReduce", mybir.AluOpType.add,
                    ins=[ar_in.opt()], outs=[ar_out.opt()],
                    replica_groups=[list(range(W))],
                )
                ar_outs.append(ar_out)

            for m in range(M_IT):
                k = m // QM
                nc.tensor.matmul(
                    wides[k][:], ohn[:, m * 128:(m + 1) * 128],
                    xbf[:, (2 * m) * D:(2 * m + 2) * D],
                    start=(m % QM == 0),
                    stop=(m % QM == QM - 1),
                )
                if m % QM == QM - 1:
                    fold(wides[k], stages[k])
                    if k == 1:
                        # first-half AR launches mid-phase-1: its ~20us
                        # collective latency hides under the remaining MMs
                        nc.vector.tensor_tensor(
                            stages[0][:], stages[0][:], stages[1][:],
                            ALU.add)
                        launch_ar(stages[0], 0)
            nc.vector.tensor_tensor(stages[2][:], stages[2][:],
                                    stages[3][:], ALU.add)
            launch_ar(stages[2], 1)
            p1_ctx.close()

            # ---- keep the PE warm across the AR tail ----
            ps_warm_ctx = contextlib.ExitStack()
            ps_warm = ps_warm_ctx.enter_context(
                tc.tile_pool(name="ps_warm", bufs=1, space="PSUM"))
            wtile = ps_warm.tile([64, D], f32, tag="warm")
            for _ in range(N_WARM):
                nc.tensor.matmul(wtile[:], identf[0:64, 0:64],
                                 stages[2][:], start=True, stop=True)
            ps_warm_ctx.close()

            nc.gpsimd.dma_start(sAx[0:64, :], ar_outs[0][:])
            nc.gpsimd.dma_start(sAx[64:128, :], ar_outs[0][:])
            nc.gpsimd.dma_start(sBx[0:64, :], ar_outs[1][:])
            nc.gpsimd.dma_start(sBx[64:128, :], ar_outs[1][:])
            nc.vector.tensor_tensor(sums_sb[:], sAx[:], sBx[:], ALU.add)

            # ---- means ----
            nc.scalar.activation(mneW[0:64, 0:D], sums_sb[0:64, :],
                                 ACT.Copy, scale=ninv[0:64, :])
            nc.scalar.activation(mneW[64:128, D:2 * D], sums_sb[64:128, :],
                                 ACT.Copy, scale=ninv[64:128, :])
            nc.scalar.activation(means_f[:], sums_sb[0:64, :], ACT.Copy,
                                 scale=inv[:])

            # ---- inter-class term (tiny, replicated) ----
            # dist2[i,j] = v_i + u_j - 2<m_i,m_j> + D*eps^2
            ps_misc = contextlib.ExitStack()
            pm = ps_misc.enter_context(
                tc.tile_pool(name="ps_misc", bufs=3, space="PSUM"))
            tpa = pm.tile([96, 64], f32, tag="misc")
            nc.tensor.transpose(tpa[:], means_f[:, 0:96], identf[0:64, 0:64])
            nc.vector.tensor_copy(A1[:], tpa[:])
            nc.scalar.mul(B1[:], tpa[:], -2.0)
            tpb = pm.tile([96, 64], f32, tag="misc")
            nc.tensor.transpose(tpb[:], means_f[:, 96:192],
                                identf[0:64, 0:64])
            nc.vector.tensor_copy(A2[:], tpb[:])
            nc.scalar.mul(B2[:], tpb[:], -2.0)
            nc.scalar.activation(rscr[:], means_f[:], ACT.Square,
                                 accum_out=rr[:])
            nc.vector.tensor_reduce(ts[:], means_f[:], mybir.AxisListType.X,
                                    ALU.add)
            nc.scalar.activation(vb[:], ts[:], ACT.Identity,
                                 bias=rr[:], scale=2.0 * EPS)
            nc.scalar.activation(uu[:], ts[:], ACT.Identity,
                                 bias=rr[:], scale=-2.0 * EPS)
            nc.vector.tensor_scalar_add(vb[:], vb[:], float(D) * EPS * EPS)
            ut_ps = pm.tile([1, 64], f32, tag="misc")
            nc.tensor.transpose(ut_ps[:], uu[:], identf[0:64, 0:64])
            nc.vector.tensor_copy(ut_sb[:], ut_ps[:])
            nc.any.memset(ones_row[:], 1.0)
            g_ps = pm.tile([64, 64], f32, tag="misc")
            nc.tensor.matmul(g_ps[:], A1[:], B1[:], start=True, stop=False)
            nc.tensor.matmul(g_ps[:], A2[:], B2[:], start=False, stop=False)
            nc.tensor.matmul(g_ps[:], ones_row[:], ut_sb[:],
                             start=False, stop=True)
            nc.vector.tensor_scalar(g_ps[:], g_ps[:], vb[:], 0.0,
                                    ALU.add, ALU.max)
            nc.scalar.activation(dists[:], g_ps[:], ACT.Sqrt)
            nc.vector.tensor_tensor(dscr[:], dists[:], offdiag[:], ALU.mult)
            nc.vector.tensor_reduce(inter_rows[:], dscr[:],
                                    mybir.AxisListType.X, ALU.add)
            inter_ps = pm.tile([1, 1], f32, tag="misc")
            nc.tensor.matmul(inter_ps[:], inter_rows[:], ones[0:64, :])

            # ---- phase 2: intra-class norms ----
            p2_ctx = contextlib.ExitStack()
            ps_diff = p2_ctx.enter_context(
                tc.tile_pool(name="ps_diff", bufs=5, space="PSUM"))
            dsq = None
            nsq = 0
            for m in range(M_IT):
                t0 = 2 * m
                p = paths[m]
                dj = ps_diff.tile([128, 2 * D], f32, tag="dj")
                nc.tensor.matmul(dj[:], oht[:, m * 128:(m + 1) * 128],
                                 mneW[:], start=True, stop=(p == 2))
                if p == 1:
                    nc.tensor.matmul(dj[:], identb[:],
                                     xbf[:, t0 * D:(t0 + 2) * D],
                                     start=False, stop=True)
                    # scalar square into a 4-pair chunk, batched DVE reduce
                    if nsq == 0:
                        dsq = scr_p.tile([128, 8, D], bf16, tag="dsq",
                                         bufs=3)
                        c0 = t0
                    nc.scalar.activation(dsq[:, nsq:nsq + 2, :], dj[:],
                                         ACT.Square)
                    nsq += 2
                    if nsq == 8:
                        nc.vector.tensor_reduce(
                            sqsum[:, c0:c0 + 8], dsq[:],
                            mybir.AxisListType.X, ALU.add)
                        nsq = 0
                else:
                    # DVE: diff = dj + x (dj holds -gathered-means),
                    # then stt square-accumulate per tile
                    dcp = scr_p.tile([128, 2 * D], bf16, tag="dcp")
                    nc.vector.scalar_tensor_tensor(
                        dcp[:], dj[:], 1.0, xbf[:, t0 * D:(t0 + 2) * D],
                        ALU.bypass, ALU.add)
                    for j in (0, 1):
                        sl = dcp[:, j * D:(j + 1) * D]
                        nc.vector.scalar_tensor_tensor(
                            sl, sl, 1.0, sl, ALU.bypass, ALU.mult,
                            accum_out=sqsum[:, t0 + j:t0 + j + 1])
            p2_ctx.close()

            # ---- finalize ----
            nc.scalar.activation(norms[:], sqsum[:], ACT.Sqrt,
                                 accum_out=nsum[:])
            intra_ps = pm.tile([1, 1], f32, tag="misc")
            nc.tensor.matmul(intra_ps[:], nsum[:], ones[:])
            nc.scalar.activation(tmp_s[:], intra_ps[:], ACT.Copy)
            # out = intra_partial - inter/W
            nc.scalar.activation(outv[:], inter_ps[:], ACT.Identity,
                                 bias=tmp_s[:], scale=-1.0 / W)
            nc.sync.dma_start(out_d.ap(), outv[:])
            ps_misc.close()

    nc.compile()
    return nc


def _consts():
    import ml_dtypes
    out = {}
    ident = np.eye(128, dtype=np.float32)
    out["identf"] = ident
    out["identb"] = ident.astype(ml_dtypes.bfloat16)
    out["ones"] = np.ones((128, 1), dtype=np.float32)
    out["offdiag"] = (1.0 - np.eye(C)).astype(np.float32)
    out["foldm"] = np.tile(np.eye(C, dtype=np.float32), (2, 1))
    return out


def kernel(logits: np.ndarray, labels: np.ndarray) -> np.ndarray:
    import sys
    if "/opt/trn_rl_repo" not in sys.path:
        sys.path.insert(0, "/opt/trn_rl_repo")
    import ml_dtypes
    from concourse import bass_utils

    if "nc" not in _COMPILED:
        _COMPILED["nc"] = _build()
    nc = _COMPILED["nc"]

    bf16 = ml_dtypes.bfloat16
    logits = np.asarray(logits, dtype=np.float32)
    labels_i = np.asarray(labels).astype(np.int64)

    counts = np.bincount(labels_i, minlength=C).astype(np.float32)
    inv = (1.0 / counts).reshape(64, 1).astype(np.float32)
    inv128 = np.tile(inv, (2, 1))
    consts = _consts()
    cls = np.arange(C, dtype=np.int64)

    in_maps = []
    for i in range(W):
        sl = slice(i * NL, (i + 1) * NL)
        lab = labels_i[sl]
        xdev = np.ascontiguousarray(
            logits[sl].reshape(T, 128, D).transpose(1, 0, 2).reshape(
                128, T * D)).astype(bf16)
        L = lab.reshape(T, 128).T                       # [128, T]
        ohn = (L[:, :, None] == cls[None, None, :]).reshape(
            128, T * C).astype(bf16)
        lab_pairs = lab.reshape(M_IT, 2, 128)           # [m, j, i]
        eq0 = (lab_pairs[None, :, 0, :] == cls[:, None, None])
        eq1 = (lab_pairs[None, :, 1, :] == cls[:, None, None])
        oht = np.ascontiguousarray(
            np.concatenate([eq0, eq1], axis=0).reshape(
                128, M_IT * 128)).astype(bf16)
        m = {
            "x": xdev,
            "ohn": np.ascontiguousarray(ohn),
            "oht": oht,
            "ninv": -inv128,
            "inv": inv,
        }
        m.update(consts)
        in_maps.append(m)

    res = bass_utils.run_bass_kernel_spmd(nc, in_maps, core_ids=list(range(W)))
    total = np.float64(0.0)
    for i in range(W):
        total += np.float64(res.results[i]["out"][0, 0])
    return np.float32(total)



# revision 2
# speedup vs baseline: 1.2665x; 1.2665x over previous
"""ClusterLoss Trainium2 kernel: 8-core data-parallel Bass/Tile implementation.

Math (C=64 classes, D=192, N=262144):
  sums[c]  = sum_{i: lab_i=c} x_i            (one-hot matmul, PSUM accumulate)
  means    = sums / counts                   (counts via host bincount)
  intra    = sum_i ||x_i - means[lab_i] + eps||_2
  inter    = sum_{i != j} ||mean_i - mean_j + eps||_2
  out      = intra - inter
8 cores shard rows; class sums are AllReduce'd; inter replicated; host
sums the 8 partial scalars.

v2 design vs baseline:
  - host supplies x in bf16 (halves HBM) and both one-hot layouts
    (normal for the sums matmul lhsT, transposed for the phase-2 gather
    lhsT) - removes all DVE one-hot builds / transposes / copies.
  - phase 2 splits work across scalar+vector+gpsimd via three paths:
      P1: gather+ident matmul -> ACT.Square -> ts-accum reduce
      P2: gather matmul only  -> DVE stt (diff) -> stt-square-accum
      P3: gather+ident matmul -> DVE ts copy   -> stt-square-accum
  - dummy matmuls keep the PE HAM-warm across the AllReduce gap.

eps note: intra additive eps shifts result ~1e-10 relative - dropped.
Inter eps kept exactly via the Gram expansion (see inter block).
"""

import contextlib

import numpy as np

N, D, C, W = 262144, 192, 64, 8
NL = N // W            # rows per core = 32768
T = NL // 128          # 128-row tiles per core = 256
M_IT = T // 2          # tile pairs = 128
EPS = 1e-6

_COMPILED = {}

# phase-2 path mix (counts over M_IT pairs) and reduce-engine choice
N_WARM = 0  # PE warm-keepers (scheduler hoists them into phase 1: off)


def _mk_paths():
    """Blocks of 28 scalar-path pairs (7 aligned 4-pair reduce chunks)
    followed by 4 DVE-path pairs, repeated 4x over the 128 pairs."""
    paths = []
    for g in range(4):
        paths += [1] * 28 + [2] * 4
    return paths


def _build(debug=False):
    import sys
    if "/opt/trn_rl_repo" not in sys.path:
        sys.path.insert(0, "/opt/trn_rl_repo")
    from concourse import bacc, tile, mybir

    f32 = mybir.dt.float32
    bf16 = mybir.dt.bfloat16
    ACT = mybir.ActivationFunctionType
    ALU = mybir.AluOpType

    nc = bacc.Bacc("TRN2", target_bir_lowering=False, debug=debug,
                   num_devices=W)

    x_d = nc.dram_tensor("x", [128, T * D], bf16, kind="ExternalInput")
    ohn_d = nc.dram_tensor("ohn", [128, M_IT * 128], bf16,
                           kind="ExternalInput")
    oht_d = nc.dram_tensor("oht", [128, M_IT * 128], bf16,
                           kind="ExternalInput")
    identb_d = nc.dram_tensor("identb", [128, 128], bf16,
                              kind="ExternalInput")
    identf_d = nc.dram_tensor("identf", [128, 128], f32,
                              kind="ExternalInput")
    ninv_d = nc.dram_tensor("ninv", [128, 1], f32, kind="ExternalInput")
    inv_d = nc.dram_tensor("inv", [64, 1], f32, kind="ExternalInput")
    ones_d = nc.dram_tensor("ones", [128, 1], f32, kind="ExternalInput")
    offdiag_d = nc.dram_tensor("offdiag", [C, C], f32, kind="ExternalInput")
    foldm_d = nc.dram_tensor("foldm", [128, C], f32, kind="ExternalInput")
    out_d = nc.dram_tensor("out", [1, 1], f32, kind="ExternalOutput")

    paths = _mk_paths()

    with tile.TileContext(nc) as tc:
        with (
            tc.tile_pool(name="singles", bufs=1) as sg,
            tc.tile_pool(name="scr", bufs=6) as scr_p,
            tc.tile_pool(name="dram", bufs=1, space="DRAM") as dram_p,
        ):
            # ---- dummy AllReduce at t=0: absorbs cross-core launch skew
            # and the CC bootstrap barrier under phase 1, so the real
            # sums AllReduces run at steady-state (~10us) latency.
            dum_sb = sg.tile([1, 1], f32, name="dum_sb")
            dum_in = dram_p.tile([1, 1], f32, tag="dumi", name="dum_in")
            dum_out = dram_p.tile([1, 1], f32, tag="dumo",
                                  addr_space="Shared", name="dum_out")
            nc.vector.memset(dum_sb[:], 1.0)
            nc.gpsimd.dma_start(dum_in[:], dum_sb[:])
            nc.gpsimd.collective_compute(
                "AllReduce", mybir.AluOpType.add,
                ins=[dum_in.opt()], outs=[dum_out.opt()],
                replica_groups=[list(range(W))],
            )
            # ---- persistent SBUF ----
            xbf = sg.tile([128, T * D], bf16)         # resident logits
            ohn = sg.tile([128, M_IT * 128], bf16)    # pair-normal one-hots
            oht = sg.tile([128, M_IT * 128], bf16)    # pair-transposed
            identb = sg.tile([128, 128], bf16)
            identf = sg.tile([128, 128], f32)
            ninv = sg.tile([128, 1], f32)
            inv = sg.tile([64, 1], f32)
            ones = sg.tile([128, 1], f32)
            offdiag = sg.tile([C, C], f32)
            foldm = sg.tile([128, C], f32)
            sums_sb = sg.tile([128, D], f32)
            mneW = sg.tile([128, 2 * D], bf16)        # block-diag -means
            means_f = sg.tile([64, D], f32)
            sqsum = sg.tile([128, T], f32)
            norms = sg.tile([128, T], bf16)
            nsum = sg.tile([128, 1], f32)
            A1 = sg.tile([96, 64], f32)
            A2 = sg.tile([96, 64], f32)
            B1 = sg.tile([96, 64], f32)
            B2 = sg.tile([96, 64], f32)
            rr = sg.tile([64, 1], f32)
            ts = sg.tile([64, 1], f32)
            vb = sg.tile([64, 1], f32)
            uu = sg.tile([64, 1], f32)
            ut_sb = sg.tile([1, 64], f32)
            ones_row = sg.tile([1, 64], f32)
            sums_stage = sg.tile([64, D], f32)
            sums_stage2 = sg.tile([64, D], f32)
            sAx = sg.tile([128, D], f32)
            sBx = sg.tile([128, D], f32)
            dists = sg.tile([64, 64], f32)
            dscr = sg.tile([64, 64], f32)
            inter_rows = sg.tile([64, 1], f32)
            rscr = sg.tile([64, D], bf16)
            wmix = sg.tile([128, D], f32)
            tmp_s = sg.tile([1, 1], f32)
            outv = sg.tile([1, 1], f32)

            # ---- input DMA: x on sync+gpsimd queues, one-hots behind ----
            # small leading chunks so pair-0 data lands early, then bulk
            xsplits = [0, 4, 8, 12, 16] + list(range(32, T + 1, 16))
            for ci in range(len(xsplits) - 1):
                lo, hi = xsplits[ci] * D, xsplits[ci + 1] * D
                q = nc.sync if ci % 2 == 0 else nc.gpsimd
                q.dma_start(xbf[:, lo:hi], x_d.ap()[:, lo:hi])
            nc.scalar.dma_start(foldm[:], foldm_d.ap())
            osplits = [0, 2, 4, 8, 16] + list(range(32, M_IT + 1, 16))
            for ci in range(len(osplits) - 1):
                lo, hi = osplits[ci] * 128, osplits[ci + 1] * 128
                nc.scalar.dma_start(ohn[:, lo:hi], ohn_d.ap()[:, lo:hi])
            nc.scalar.dma_start(identb[:], identb_d.ap())
            nc.scalar.dma_start(identf[:], identf_d.ap())
            nc.scalar.dma_start(ninv[:], ninv_d.ap())
            nc.scalar.dma_start(inv[:], inv_d.ap())
            nc.scalar.dma_start(ones[:], ones_d.ap())
            nc.scalar.dma_start(offdiag[:], offdiag_d.ap())
            # oht is needed only in phase 2: queue it behind ohn on the
            # scalar ring (keeps the gpsimd ring free for the AR copies).
            OCH = 16 * 128
            for ci in range(M_IT // 16):
                lo, hi = ci * OCH, (ci + 1) * OCH
                nc.scalar.dma_start(oht[:, lo:hi], oht_d.ap()[:, lo:hi])
            nc.vector.memset(mneW[:], 0.0)

            # ---- phase 1: class sums via one-hot matmuls ----
            # lhsT = ohn pair [128,128] (cols = 2x64 classes), rhs = x pair
            # [128,384]. Diagonal blocks of the wide PSUM are the two
            # partial sums; fold extracts+adds them.
            p1_ctx = contextlib.ExitStack()
            ps_sums = p1_ctx.enter_context(
                tc.tile_pool(name="ps_sums", bufs=1, space="PSUM"))
            ps_fold = p1_ctx.enter_context(
                tc.tile_pool(name="ps_fold", bufs=2, space="PSUM"))
            QM = M_IT // 4
            wides = [ps_sums.tile([128, 2 * D], f32, tag=f"w{k}",
                                  name=f"wide{k}")
                     for k in range(4)]
            stages = [sums_stage, sums_stage2,
                      sg.tile([64, D], f32, name="sums_stage3"),
                      sg.tile([64, D], f32, name="sums_stage4")]

            def fold(wide, stage_out):
                nc.vector.tensor_copy(wmix[0:64, :], wide[0:64, 0:D])
                nc.vector.tensor_copy(wmix[64:128, :], wide[64:128, D:2 * D])
                fps = ps_fold.tile([64, D], f32, tag="fold")
                nc.tensor.matmul(fps[:], foldm[:], wmix[:],
                                 start=True, stop=True)
                nc.vector.tensor_copy(stage_out[:], fps[:])

            ar_outs = []

            def launch_ar(src, k):
                ar_in = dram_p.tile([64, D], f32, tag=f"ar{k}i",
                                    name=f"arin{k}")
                ar_out = dram_p.tile([64, D], f32, tag=f"ar{k}o",
                                     addr_space="Shared", name=f"arout{k}")
                nc.gpsimd.dma_start(ar_in[:], src[:])
                nc.gpsimd.collective_compute(
                    "AllReduce", mybir.AluOpType.add,
                    ins=[ar_in.opt()], outs=[ar_out.opt()],
                    replica_groups=[list(range(W))],
                )
                ar_outs.append(ar_out)

            for m in range(M_IT):
                k = m // QM
                nc.tensor.matmul(
                    wides[k][:], ohn[:, m * 128:(m + 1) * 128],
                    xbf[:, (2 * m) * D:(2 * m + 2) * D],
                    start=(m % QM == 0),
                    stop=(m % QM == QM - 1),
                )
                if m % QM == QM - 1:
                    fold(wides[k], stages[k])
                    if k == 1:
                        # first-half AR launches mid-phase-1: its ~20us
                        # collective latency hides under the remaining MMs
                        nc.vector.tensor_tensor(
                            stages[0][:], stages[0][:], stages[1][:],
                            ALU.add)
                        launch_ar(stages[0], 0)
            nc.vector.tensor_tensor(stages[2][:], stages[2][:],
                                    stages[3][:], ALU.add)
            launch_ar(stages[2], 1)
            p1_ctx.close()

            # ---- keep the PE warm across the AR tail ----
            ps_warm_ctx = contextlib.ExitStack()
            ps_warm = ps_warm_ctx.enter_context(
                tc.tile_pool(name="ps_warm", bufs=1, space="PSUM"))
            wtile = ps_warm.tile([64, D], f32, tag="warm")
            for _ in range(N_WARM):
                nc.tensor.matmul(wtile[:], identf[0:64, 0:64],
                                 stages[2][:], start=True, stop=True)
            ps_warm_ctx.close()

            nc.gpsimd.dma_start(sAx[0:64, :], ar_outs[0][:])
            nc.gpsimd.dma_start(sAx[64:128, :], ar_outs[0][:])
            nc.gpsimd.dma_start(sBx[0:64, :], ar_outs[1][:])
            nc.gpsimd.dma_start(sBx[64:128, :], ar_outs[1][:])
            nc.vector.tensor_tensor(sums_sb[:], sAx[:], sBx[:], ALU.add)

            # ---- means ----
            nc.scalar.activation(mneW[0:64, 0:D], sums_sb[0:64, :],
                                 ACT.Copy, scale=ninv[0:64, :])
            nc.scalar.activation(mneW[64:128, D:2 * D], sums_sb[64:128, :],
                                 ACT.Copy, scale=ninv[64:128, :])
            nc.scalar.activation(means_f[:], sums_sb[0:64, :], ACT.Copy,
                                 scale=inv[:])

            # ---- inter-class term (tiny, replicated) ----
            # dist2[i,j] = v_i + u_j - 2<m_i,m_j> + D*eps^2
            ps_misc = contextlib.ExitStack()
            pm = ps_misc.enter_context(
                tc.tile_pool(name="ps_misc", bufs=3, space="PSUM"))
            tpa = pm.tile([96, 64], f32, tag="misc")
            nc.tensor.transpose(tpa[:], means_f[:, 0:96], identf[0:64, 0:64])
            nc.vector.tensor_copy(A1[:], tpa[:])
            nc.scalar.mul(B1[:], tpa[:], -2.0)
            tpb = pm.tile([96, 64], f32, tag="misc")
            nc.tensor.transpose(tpb[:], means_f[:, 96:192],
                                identf[0:64, 0:64])
            nc.vector.tensor_copy(A2[:], tpb[:])
            nc.scalar.mul(B2[:], tpb[:], -2.0)
            nc.scalar.activation(rscr[:], means_f[:], ACT.Square,
                                 accum_out=rr[:])
            nc.vector.tensor_reduce(ts[:], means_f[:], mybir.AxisListType.X,
                                    ALU.add)
            nc.scalar.activation(vb[:], ts[:], ACT.Identity,
                                 bias=rr[:], scale=2.0 * EPS)
            nc.scalar.activation(uu[:], ts[:], ACT.Identity,
                                 bias=rr[:], scale=-2.0 * EPS)
            nc.vector.tensor_scalar_add(vb[:], vb[:], float(D) * EPS * EPS)
            ut_ps = pm.tile([1, 64], f32, tag="misc")
            nc.tensor.transpose(ut_ps[:], uu[:], identf[0:64, 0:64])
            nc.vector.tensor_copy(ut_sb[:], ut_ps[:])
            nc.any.memset(ones_row[:], 1.0)
            g_ps = pm.tile([64, 64], f32, tag="misc")
            nc.tensor.matmul(g_ps[:], A1[:], B1[:], start=True, stop=False)
            nc.tensor.matmul(g_ps[:], A2[:], B2[:], start=False, stop=False)
            nc.tensor.matmul(g_ps[:], ones_row[:], ut_sb[:],
                             start=False, stop=True)
            nc.vector.tensor_scalar(g_ps[:], g_ps[:], vb[:], 0.0,
                                    ALU.add, ALU.max)
            nc.scalar.activation(dists[:], g_ps[:], ACT.Sqrt)
            nc.vector.tensor_tensor(dscr[:], dists[:], offdiag[:], ALU.mult)
            nc.vector.tensor_reduce(inter_rows[:], dscr[:],
                                    mybir.AxisListType.X, ALU.add)
            inter_ps = pm.tile([1, 1], f32, tag="misc")
            nc.tensor.matmul(inter_ps[:], inter_rows[:], ones[0:64, :])

            # ---- phase 2: intra-class norms ----
            p2_ctx = contextlib.ExitStack()
            ps_diff = p2_ctx.enter_context(
                tc.tile_pool(name="ps_diff", bufs=5, space="PSUM"))
            dsq = None
            nsq = 0
            for m in range(M_IT):
                t0 = 2 * m
                p = paths[m]
                dj = ps_diff.tile([128, 2 * D], f32, tag="dj")
                nc.tensor.matmul(dj[:], oht[:, m * 128:(m + 1) * 128],
                                 mneW[:], start=True, stop=(p == 2))
                if p == 1:
                    nc.tensor.matmul(dj[:], identb[:],
                                     xbf[:, t0 * D:(t0 + 2) * D],
                                     start=False, stop=True)
                    # scalar square into a 4-pair chunk, batched DVE reduce
                    if nsq == 0:
                        dsq = scr_p.tile([128, 8, D], bf16, tag="dsq",
                                         bufs=3)
                        c0 = t0
                    nc.scalar.activation(dsq[:, nsq:nsq + 2, :], dj[:],
                                         ACT.Square)
                    nsq += 2
                    if nsq == 8:
                        nc.vector.tensor_reduce(
                            sqsum[:, c0:c0 + 8], dsq[:],
                            mybir.AxisListType.X, ALU.add)
                        nsq = 0
                else:
                    # DVE: diff = dj + x (dj holds -gathered-means),
                    # then stt square-accumulate per tile
                    dcp = scr_p.tile([128, 2 * D], bf16, tag="dcp")
                    nc.vector.scalar_tensor_tensor(
                        dcp[:], dj[:], 1.0, xbf[:, t0 * D:(t0 + 2) * D],
                        ALU.bypass, ALU.add)
                    for j in (0, 1):
                        sl = dcp[:, j * D:(j + 1) * D]
                        nc.vector.scalar_tensor_tensor(
                            sl, sl, 1.0, sl, ALU.bypass, ALU.mult,
                            accum_out=sqsum[:, t0 + j:t0 + j + 1])
            p2_ctx.close()

            # ---- finalize ----
            nc.scalar.activation(norms[:], sqsum[:], ACT.Sqrt,
                                 accum_out=nsum[:])
            intra_ps = pm.tile([1, 1], f32, tag="misc")
            nc.tensor.matmul(intra_ps[:], nsum[:], ones[:])
            nc.scalar.activation(tmp_s[:], intra_ps[:], ACT.Copy)
            # out = intra_partial - inter/W
            nc.scalar.activation(outv[:], inter_ps[:], ACT.Identity,
                                 bias=tmp_s[:], scale=-1.0 / W)
            nc.sync.dma_start(out_d.ap(), outv[:])
            ps_misc.close()

    nc.compile()
    return nc


def _consts():
    import ml_dtypes
    out = {}
    ident = np.eye(128, dtype=np.float32)
    out["identf"] = ident
    out["identb"] = ident.astype(ml_dtypes.bfloat16)
    out["ones"] = np.ones((128, 1), dtype=np.float32)
    out["offdiag"] = (1.0 - np.eye(C)).astype(np.float32)
    out["foldm"] = np.tile(np.eye(C, dtype=np.float32), (2, 1))
    return out


def kernel(logits: np.ndarray, labels: np.ndarray) -> np.ndarray:
    import sys
    if "/opt/trn_rl_repo" not in sys.path:
        sys.path.insert(0, "/opt/trn_rl_repo")
    import ml_dtypes
    from concourse import bass_utils

    if "nc" not in _COMPILED:
        _COMPILED["nc"] = _build()
    nc = _COMPILED["nc"]

    bf16 = ml_dtypes.bfloat16
    logits = np.asarray(logits, dtype=np.float32)
    labels_i = np.asarray(labels).astype(np.int64)

    counts = np.bincount(labels_i, minlength=C).astype(np.float32)
    inv = (1.0 / counts).reshape(64, 1).astype(np.float32)
    inv128 = np.tile(inv, (2, 1))
    consts = _consts()
    cls = np.arange(C, dtype=np.int64)

    in_maps = []
    for i in range(W):
        sl = slice(i * NL, (i + 1) * NL)
        lab = labels_i[sl]
        xdev = np.ascontiguousarray(
            logits[sl].reshape(T, 128, D).transpose(1, 0, 2).reshape(
                128, T * D)).astype(bf16)
        L = lab.reshape(T, 128).T                       # [128, T]
        ohn = (L[:, :, None] == cls[None, None, :]).reshape(
            128, T * C).astype(bf16)
        lab_pairs = lab.reshape(M_IT, 2, 128)           # [m, j, i]
        eq0 = (lab_pairs[None, :, 0, :] == cls[:, None, None])
        eq1 = (lab_pairs[None, :, 1, :] == cls[:, None, None])
        oht = np.ascontiguousarray(
            np.concatenate([eq0, eq1], axis=0).reshape(
                128, M_IT * 128)).astype(bf16)
        m = {
            "x": xdev,
            "ohn": np.ascontiguousarray(ohn),
            "oht": oht,
            "ninv": -inv128,
            "inv": inv,
        }
        m.update(consts)
        in_maps.append(m)

    res = bass_utils.run_bass_kernel_spmd(nc, in_maps, core_ids=list(range(W)))
    total = np.float64(0.0)
    for i in range(W):
        total += np.float64(res.results[i]["out"][0, 0])
    return np.float32(total)



# revision 6
# speedup vs baseline: 1.2831x; 1.0130x over previous
"""ClusterLoss Trainium2 kernel: 8-core data-parallel Bass/Tile implementation.

Math (C=64 classes, D=192, N=262144):
  sums[c]  = sum_{i: lab_i=c} x_i            (one-hot matmul, PSUM accumulate)
  means    = sums / counts                   (counts via host bincount)
  intra    = sum_i ||x_i - means[lab_i] + eps||_2
  inter    = sum_{i != j} ||mean_i - mean_j + eps||_2
  out      = intra - inter
8 cores shard rows; class sums are AllReduce'd; inter replicated; host
sums the 8 partial scalars.

v2 design vs baseline:
  - host supplies x in bf16 (halves HBM) and both one-hot layouts
    (normal for the sums matmul lhsT, transposed for the phase-2 gather
    lhsT) - removes all DVE one-hot builds / transposes / copies.
  - phase 2 splits work across scalar+vector+gpsimd via three paths:
      P1: gather+ident matmul -> ACT.Square -> ts-accum reduce
      P2: gather matmul only  -> DVE stt (diff) -> stt-square-accum
      P3: gather+ident matmul -> DVE ts copy   -> stt-square-accum
  - dummy matmuls keep the PE HAM-warm across the AllReduce gap.

eps note: intra additive eps shifts result ~1e-10 relative - dropped.
Inter eps kept exactly via the Gram expansion (see inter block).
"""

import contextlib

import numpy as np

N, D, C, W = 262144, 192, 64, 8
NL = N // W            # rows per core = 32768
T = NL // 128          # 128-row tiles per core = 256
M_IT = T // 2          # tile pairs = 128
EPS = 1e-6

_COMPILED = {}

# phase-2 path mix (counts over M_IT pairs) and reduce-engine choice
N_WARM = 0  # PE warm-keepers (scheduler hoists them into phase 1: off)


def _mk_paths():
    """Blocks of 28 scalar-path pairs (7 aligned 4-pair reduce chunks)
    followed by 4 DVE-path pairs, repeated 4x over the 128 pairs."""
    paths = []
    for g in range(4):
        paths += [1] * 28 + [2] * 4
    return paths


def _build(debug=False):
    import sys
    if "/opt/trn_rl_repo" not in sys.path:
        sys.path.insert(0, "/opt/trn_rl_repo")
    from concourse import bacc, tile, mybir

    f32 = mybir.dt.float32
    bf16 = mybir.dt.bfloat16
    ACT = mybir.ActivationFunctionType
    ALU = mybir.AluOpType

    nc = bacc.Bacc("TRN2", target_bir_lowering=False, debug=debug,
                   num_devices=W)

    x_d = nc.dram_tensor("x", [128, T * D], bf16, kind="ExternalInput")
    ohn_d = nc.dram_tensor("ohn", [128, M_IT * 128], bf16,
                           kind="ExternalInput")
    oht_d = nc.dram_tensor("oht", [128, M_IT * 128], bf16,
                           kind="ExternalInput")
    identb_d = nc.dram_tensor("identb", [128, 128], bf16,
                              kind="ExternalInput")
    identf_d = nc.dram_tensor("identf", [128, 128], f32,
                              kind="ExternalInput")
    ninv_d = nc.dram_tensor("ninv", [128, 1], f32, kind="ExternalInput")
    inv_d = nc.dram_tensor("inv", [64, 1], f32, kind="ExternalInput")
    ones_d = nc.dram_tensor("ones", [128, 1], f32, kind="ExternalInput")
    offdiag_d = nc.dram_tensor("offdiag", [C, C], f32, kind="ExternalInput")
    foldm_d = nc.dram_tensor("foldm", [128, C], f32, kind="ExternalInput")
    out_d = nc.dram_tensor("out", [1, 1], f32, kind="ExternalOutput")

    paths = _mk_paths()

    with tile.TileContext(nc) as tc:
        with (
            tc.tile_pool(name="singles", bufs=1) as sg,
            tc.tile_pool(name="scr", bufs=6) as scr_p,
            tc.tile_pool(name="dram", bufs=1, space="DRAM") as dram_p,
        ):
            # ---- dummy AllReduce at t=0: absorbs cross-core launch skew
            # and the CC bootstrap barrier under phase 1, so the real
            # sums AllReduces run at steady-state (~10us) latency.
            dum_in = dram_p.tile([1, 1], f32, tag="dumi", name="dum_in")
            dum_out = dram_p.tile([1, 1], f32, tag="dumo",
                                  addr_space="Shared", name="dum_out")
            nc.gpsimd.collective_compute(
                "AllReduce", mybir.AluOpType.add,
                ins=[dum_in.opt()], outs=[dum_out.opt()],
                replica_groups=[list(range(W))],
            )
            # ---- persistent SBUF ----
            xbf = sg.tile([128, T * D], bf16)         # resident logits
            ohn = sg.tile([128, M_IT * 128], bf16)    # pair-normal one-hots
            oht = sg.tile([128, M_IT * 128], bf16)    # pair-transposed
            identb = sg.tile([128, 128], bf16)
            identf = sg.tile([128, 128], f32)
            ninv = sg.tile([128, 1], f32)
            inv = sg.tile([64, 1], f32)
            ones = sg.tile([128, 1], f32)
            offdiag = sg.tile([C, C], f32)
            foldm = sg.tile([128, C], f32)
            sums_sb = sg.tile([128, D], f32)
            mneW = sg.tile([128, 2 * D], bf16)        # block-diag -means
            means_f = sg.tile([64, D], f32)
            sqsum = sg.tile([128, T], f32)
            norms = sg.tile([128, T], bf16)
            nsum = sg.tile([128, 1], f32)
            A1 = sg.tile([96, 64], f32)
            A2 = sg.tile([96, 64], f32)
            B1 = sg.tile([96, 64], f32)
            B2 = sg.tile([96, 64], f32)
            rr = sg.tile([64, 1], f32)
            ts = sg.tile([64, 1], f32)
            vb = sg.tile([64, 1], f32)
            uu = sg.tile([64, 1], f32)
            ut_sb = sg.tile([1, 64], f32)
            ones_row = sg.tile([1, 64], f32)
            sums_stage = sg.tile([64, D], f32)
            sums_stage2 = sg.tile([64, D], f32)
            sAx = sg.tile([128, D], f32)
            sBx = sg.tile([128, D], f32)
            dists = sg.tile([64, 64], f32)
            dscr = sg.tile([64, 64], f32)
            inter_rows = sg.tile([64, 1], f32)
            rscr = sg.tile([64, D], bf16)
            wmix = sg.tile([128, D], f32)
            tmp_s = sg.tile([1, 1], f32)
            outv = sg.tile([1, 1], f32)

            # ---- input DMA: x on sync+gpsimd queues, one-hots behind ----
            # small leading chunks so pair-0 data lands early, then bulk
            xsplits = [0, 4, 8, 12, 16] + list(range(32, T + 1, 16))
            for ci in range(len(xsplits) - 1):
                lo, hi = xsplits[ci] * D, xsplits[ci + 1] * D
                q = nc.sync if ci % 2 == 0 else nc.gpsimd
                q.dma_start(xbf[:, lo:hi], x_d.ap()[:, lo:hi])
            nc.scalar.dma_start(foldm[:], foldm_d.ap())
            osplits = [0, 2, 4, 8, 16] + list(range(32, M_IT + 1, 16))
            for ci in range(len(osplits) - 1):
                lo, hi = osplits[ci] * 128, osplits[ci + 1] * 128
                nc.scalar.dma_start(ohn[:, lo:hi], ohn_d.ap()[:, lo:hi])
            nc.scalar.dma_start(identb[:], identb_d.ap())
            nc.scalar.dma_start(identf[:], identf_d.ap())
            nc.scalar.dma_start(ninv[:], ninv_d.ap())
            nc.scalar.dma_start(inv[:], inv_d.ap())
            nc.scalar.dma_start(ones[:], ones_d.ap())
            nc.scalar.dma_start(offdiag[:], offdiag_d.ap())
            # oht is needed only in phase 2: queue it behind ohn on the
            # scalar ring (keeps the gpsimd ring free for the AR copies).
            OCH = 16 * 128
            for ci in range(M_IT // 16):
                lo, hi = ci * OCH, (ci + 1) * OCH
                nc.scalar.dma_start(oht[:, lo:hi], oht_d.ap()[:, lo:hi])
            nc.vector.memset(mneW[:], 0.0)

            # ---- phase 1: class sums via one-hot matmuls ----
            # lhsT = ohn pair [128,128] (cols = 2x64 classes), rhs = x pair
            # [128,384]. Diagonal blocks of the wide PSUM are the two
            # partial sums; fold extracts+adds them.
            p1_ctx = contextlib.ExitStack()
            ps_sums = p1_ctx.enter_context(
                tc.tile_pool(name="ps_sums", bufs=1, space="PSUM"))
            ps_fold = p1_ctx.enter_context(
                tc.tile_pool(name="ps_fold", bufs=2, space="PSUM"))
            QM = M_IT // 4
            wides = [ps_sums.tile([128, 2 * D], f32, tag=f"w{k}",
                                  name=f"wide{k}")
                     for k in range(4)]
            stages = [sums_stage, sums_stage2,
                      sg.tile([64, D], f32, name="sums_stage3"),
                      sg.tile([64, D], f32, name="sums_stage4")]

            def fold(wide, stage_out):
                nc.vector.tensor_copy(wmix[0:64, :], wide[0:64, 0:D])
                nc.vector.tensor_copy(wmix[64:128, :], wide[64:128, D:2 * D])
                fps = ps_fold.tile([64, D], f32, tag="fold")
                nc.tensor.matmul(fps[:], foldm[:], wmix[:],
                                 start=True, stop=True)
                nc.vector.tensor_copy(stage_out[:], fps[:])

            ar_outs = []

            def launch_ar(src, k):
                ar_in = dram_p.tile([64, D], f32, tag=f"ar{k}i",
                                    name=f"arin{k}")
                ar_out = dram_p.tile([64, D], f32, tag=f"ar{k}o",
                                     addr_space="Shared", name=f"arout{k}")
                nc.gpsimd.dma_start(ar_in[:], src[:])
                nc.gpsimd.collective_compute(
                    "AllReduce", mybir.AluOpType.add,
                    ins=[ar_in.opt()], outs=[ar_out.opt()],
                    replica_groups=[list(range(W))],
                )
                ar_outs.append(ar_out)

            for m in range(M_IT):
                k = m // QM
                nc.tensor.matmul(
                    wides[k][:], ohn[:, m * 128:(m + 1) * 128],
                    xbf[:, (2 * m) * D:(2 * m + 2) * D],
                    start=(m % QM == 0),
                    stop=(m % QM == QM - 1),
                )
                if m % QM == QM - 1:
                    fold(wides[k], stages[k])
                    if k == 1:
                        # first-half AR launches mid-phase-1: its ~20us
                        # collective latency hides under the remaining MMs
                        nc.vector.tensor_tensor(
                            stages[0][:], stages[0][:], stages[1][:],
                            ALU.add)
                        launch_ar(stages[0], 0)
            nc.vector.tensor_tensor(stages[2][:], stages[2][:],
                                    stages[3][:], ALU.add)
            launch_ar(stages[2], 1)
            p1_ctx.close()

            # ---- keep the PE warm across the AR tail ----
            ps_warm_ctx = contextlib.ExitStack()
            ps_warm = ps_warm_ctx.enter_context(
                tc.tile_pool(name="ps_warm", bufs=1, space="PSUM"))
            wtile = ps_warm.tile([64, D], f32, tag="warm")
            for _ in range(N_WARM):
                nc.tensor.matmul(wtile[:], identf[0:64, 0:64],
                                 stages[2][:], start=True, stop=True)
            ps_warm_ctx.close()

            nc.gpsimd.dma_start(sAx[0:64, :], ar_outs[0][:])
            nc.gpsimd.dma_start(sAx[64:128, :], ar_outs[0][:])
            nc.gpsimd.dma_start(sBx[0:64, :], ar_outs[1][:])
            nc.gpsimd.dma_start(sBx[64:128, :], ar_outs[1][:])
            nc.vector.tensor_tensor(sums_sb[:], sAx[:], sBx[:], ALU.add)

            # ---- means ----
            nc.scalar.activation(mneW[0:64, 0:D], sums_sb[0:64, :],
                                 ACT.Copy, scale=ninv[0:64, :])
            nc.scalar.activation(mneW[64:128, D:2 * D], sums_sb[64:128, :],
                                 ACT.Copy, scale=ninv[64:128, :])
            nc.scalar.activation(means_f[:], sums_sb[0:64, :], ACT.Copy,
                                 scale=inv[:])

            # ---- inter-class term (tiny, replicated) ----
            # dist2[i,j] = v_i + u_j - 2<m_i,m_j> + D*eps^2
            ps_misc = contextlib.ExitStack()
            pm = ps_misc.enter_context(
                tc.tile_pool(name="ps_misc", bufs=3, space="PSUM"))
            tpa = pm.tile([96, 64], f32, tag="misc")
            nc.tensor.transpose(tpa[:], means_f[:, 0:96], identf[0:64, 0:64])
            nc.vector.tensor_copy(A1[:], tpa[:])
            nc.scalar.mul(B1[:], tpa[:], -2.0)
            tpb = pm.tile([96, 64], f32, tag="misc")
            nc.tensor.transpose(tpb[:], means_f[:, 96:192],
                                identf[0:64, 0:64])
            nc.vector.tensor_copy(A2[:], tpb[:])
            nc.scalar.mul(B2[:], tpb[:], -2.0)
            nc.scalar.activation(rscr[:], means_f[:], ACT.Square,
                                 accum_out=rr[:])
            nc.vector.tensor_reduce(ts[:], means_f[:], mybir.AxisListType.X,
                                    ALU.add)
            nc.scalar.activation(vb[:], ts[:], ACT.Identity,
                                 bias=rr[:], scale=2.0 * EPS)
            nc.scalar.activation(uu[:], ts[:], ACT.Identity,
                                 bias=rr[:], scale=-2.0 * EPS)
            nc.vector.tensor_scalar_add(vb[:], vb[:], float(D) * EPS * EPS)
            ut_ps = pm.tile([1, 64], f32, tag="misc")
            nc.tensor.transpose(ut_ps[:], uu[:], identf[0:64, 0:64])
            nc.vector.tensor_copy(ut_sb[:], ut_ps[:])
            nc.any.memset(ones_row[:], 1.0)
            g_ps = pm.tile([64, 64], f32, tag="misc")
            nc.tensor.matmul(g_ps[:], A1[:], B1[:], start=True, stop=False)
            nc.tensor.matmul(g_ps[:], A2[:], B2[:], start=False, stop=False)
            nc.tensor.matmul(g_ps[:], ones_row[:], ut_sb[:],
                             start=False, stop=True)
            nc.vector.tensor_scalar(g_ps[:], g_ps[:], vb[:], 0.0,
                                    ALU.add, ALU.max)
            nc.scalar.activation(dists[:], g_ps[:], ACT.Sqrt)
            nc.vector.tensor_tensor(dscr[:], dists[:], offdiag[:], ALU.mult)
            nc.vector.tensor_reduce(inter_rows[:], dscr[:],
                                    mybir.AxisListType.X, ALU.add)
            inter_ps = pm.tile([1, 1], f32, tag="misc")
            nc.tensor.matmul(inter_ps[:], inter_rows[:], ones[0:64, :])

            # ---- phase 2: intra-class norms ----
            p2_ctx = contextlib.ExitStack()
            ps_diff = p2_ctx.enter_context(
                tc.tile_pool(name="ps_diff", bufs=5, space="PSUM"))
            dsq = None
            nsq = 0
            for m in range(M_IT):
                t0 = 2 * m
                p = paths[m]
                dj = ps_diff.tile([128, 2 * D], f32, tag="dj")
                nc.tensor.matmul(dj[:], oht[:, m * 128:(m + 1) * 128],
                                 mneW[:], start=True, stop=(p == 2))
                if p == 1:
                    nc.tensor.matmul(dj[:], identb[:],
                                     xbf[:, t0 * D:(t0 + 2) * D],
                                     start=False, stop=True)
                    # scalar square into a 4-pair chunk, batched DVE reduce
                    if nsq == 0:
                        dsq = scr_p.tile([128, 8, D], bf16, tag="dsq",
                                         bufs=3)
                        c0 = t0
                    nc.scalar.activation(dsq[:, nsq:nsq + 2, :], dj[:],
                                         ACT.Square)
                    nsq += 2
                    if nsq == 8:
                        nc.vector.tensor_reduce(
                            sqsum[:, c0:c0 + 8], dsq[:],
                            mybir.AxisListType.X, ALU.add)
                        nsq = 0
                else:
                    # DVE: diff = dj + x (dj holds -gathered-means),
                    # then stt square-accumulate per tile
                    dcp = scr_p.tile([128, 2 * D], bf16, tag="dcp")
                    nc.vector.scalar_tensor_tensor(
                        dcp[:], dj[:], 1.0, xbf[:, t0 * D:(t0 + 2) * D],
                        ALU.bypass, ALU.add)
                    for j in (0, 1):
                        sl = dcp[:, j * D:(j + 1) * D]
                        nc.vector.scalar_tensor_tensor(
                            sl, sl, 1.0, sl, ALU.bypass, ALU.mult,
                            accum_out=sqsum[:, t0 + j:t0 + j + 1])
            p2_ctx.close()

            # ---- finalize ----
            nc.scalar.activation(norms[:], sqsum[:], ACT.Sqrt,
                                 accum_out=nsum[:])
            intra_ps = pm.tile([1, 1], f32, tag="misc")
            nc.tensor.matmul(intra_ps[:], nsum[:], ones[:])
            nc.scalar.activation(tmp_s[:], intra_ps[:], ACT.Copy)
            # out = intra_partial - inter/W
            nc.scalar.activation(outv[:], inter_ps[:], ACT.Identity,
                                 bias=tmp_s[:], scale=-1.0 / W)
            nc.sync.dma_start(out_d.ap(), outv[:])
            ps_misc.close()

    nc.compile()
    return nc


def _consts():
    import ml_dtypes
    out = {}
    ident = np.eye(128, dtype=np.float32)
    out["identf"] = ident
    out["identb"] = ident.astype(ml_dtypes.bfloat16)
    out["ones"] = np.ones((128, 1), dtype=np.float32)
    out["offdiag"] = (1.0 - np.eye(C)).astype(np.float32)
    out["foldm"] = np.tile(np.eye(C, dtype=np.float32), (2, 1))
    return out


def kernel(logits: np.ndarray, labels: np.ndarray) -> np.ndarray:
    import sys
    if "/opt/trn_rl_repo" not in sys.path:
        sys.path.insert(0, "/opt/trn_rl_repo")
    import ml_dtypes
    from concourse import bass_utils

    if "nc" not in _COMPILED:
        _COMPILED["nc"] = _build()
    nc = _COMPILED["nc"]

    bf16 = ml_dtypes.bfloat16
    logits = np.asarray(logits, dtype=np.float32)
    labels_i = np.asarray(labels).astype(np.int64)

    counts = np.bincount(labels_i, minlength=C).astype(np.float32)
    inv = (1.0 / counts).reshape(64, 1).astype(np.float32)
    inv128 = np.tile(inv, (2, 1))
    consts = _consts()
    cls = np.arange(C, dtype=np.int64)

    in_maps = []
    for i in range(W):
        sl = slice(i * NL, (i + 1) * NL)
        lab = labels_i[sl]
        xdev = np.ascontiguousarray(
            logits[sl].reshape(T, 128, D).transpose(1, 0, 2).reshape(
                128, T * D)).astype(bf16)
        L = lab.reshape(T, 128).T                       # [128, T]
        ohn = (L[:, :, None] == cls[None, None, :]).reshape(
            128, T * C).astype(bf16)
        lab_pairs = lab.reshape(M_IT, 2, 128)           # [m, j, i]
        eq0 = (lab_pairs[None, :, 0, :] == cls[:, None, None])
        eq1 = (lab_pairs[None, :, 1, :] == cls[:, None, None])
        oht = np.ascontiguousarray(
            np.concatenate([eq0, eq1], axis=0).reshape(
                128, M_IT * 128)).astype(bf16)
        m = {
            "x": xdev,
            "ohn": np.ascontiguousarray(ohn),
            "oht": oht,
            "ninv": -inv128,
            "inv": inv,
        }
        m.update(consts)
        in_maps.append(m)

    res = bass_utils.run_bass_kernel_spmd(nc, in_maps, core_ids=list(range(W)))
    total = np.float64(0.0)
    for i in range(W):
        total += np.float64(res.results[i]["out"][0, 0])
    return np.float32(total)

